# revision 1
# baseline (speedup 1.0000x reference)
"""KV-cache scatter update kernel for Trainium2 (8 NeuronCores).

Problem: kv_cache (2L=4, B=8, H=8, S=4096, D=128) f32, new_kv (L=2, 2, B=8,
H=8, 1, D=128) f32, position_ids (B=8, 1) int. Output = kv_cache with
new_kv[l, kv, b, h, 0, :] written at [2l+kv, b, h, pos[b], :].

Sharding: split on the H dim (size 8) across the 8 cores. Each core:
  - copies its (4, 8, 4096, 128) cache slice DRAM->DRAM (67.1 MB) in 8
    chunks of 8.39 MB alternating across the two HWDGE rings
    (qSPDynamicHW + qActDynamicHW), each chunk lowered to 64 KB
    descriptors spread over the 16 SDMA engines
  - scatters its 32 new rows (one per (layer', batch)) at runtime offsets
    computed on-device from position_ids via indirect DMAs. Copy chunk c
    contains exactly the target rows of new rows [4c, 4c+4) (row p lands
    at p*S + pos[p%8], inside the chunk's row range), so each 4-row
    scatter waits only on its own chunk and overlaps the remaining copy;
    only the last chunk's scatter sits on the critical path.

With all 8 cores running concurrently the copy moves ~330 GB/s of r+w HBM
traffic per core, ~92% of the 358 GB/s per-NC share (2 NCs per 716 GB/s
HBM stack) -> ~405-410 us per core vs the 375 us bandwidth floor.
"""

import numpy as np

import concourse.bacc as bacc
import concourse.bass as bass
import concourse.mybir as mybir
import concourse.tile as tile
from concourse.bass_utils import run_bass_kernel_spmd
from concourse.tile import add_dep_helper

L = 2
B = 8
H = 8
S = 4096
D = 128
NCORES = 8
LP = 2 * L          # 4 "layers" in the output (k/v interleaved)
ROWS = LP * B * S   # 131072 rows of D floats per core
NEW = LP * B        # 32 scattered rows per core

_NC_CACHE = {}


def _build(chain_k: int = 1, n_chunks: int = 8):
    """Build the Bass module (one NEFF, same program on all 8 cores).

    chain_k > 1 builds the same per-iteration body (bulk copy + scatter)
    repeated and dependency-chained K times — used only by the timing
    harness to measure steady-state per-iteration HW time via the slope
    method (dispatch overhead cancels).
    """
    global _NC_CACHE
    key = (chain_k, n_chunks)
    if key in _NC_CACHE:
        return _NC_CACHE[key]

    nc = bacc.Bacc(
        "TRN2",
        target_bir_lowering=False,
        debug=False,
        num_devices=NCORES,
    )
    rpc = NEW // n_chunks          # scattered rows per copy chunk
    rows_pc = ROWS // n_chunks     # flat cache rows per copy chunk
    assert rpc * S == rows_pc and rpc >= 2

    kv = nc.dram_tensor("kv", [ROWS, D], mybir.dt.float32, kind="ExternalInput")
    newkv = nc.dram_tensor("newkv", [NEW, D], mybir.dt.float32, kind="ExternalInput")
    # pos[q, c] = position of scattered row p = c*rpc + q (pre-arranged on
    # host). Chunk c's indices then live in COLUMN c at partitions 0..rpc-1:
    # the indirect-DMA ucode honors the offset AP's byte offset but ignores
    # its partition offset, so a row-sliced [NEW, 1] index tile silently
    # reads partition 0's indices for every chunk.
    pos = nc.dram_tensor("pos", [rpc, n_chunks], mybir.dt.int32, kind="ExternalInput")
    out = nc.dram_tensor("out", [ROWS, D], mybir.dt.float32, kind="ExternalOutput")

    with tile.TileContext(nc) as tc:
        with tc.tile_pool(name="sb", bufs=1) as pool:
            newt = pool.tile([NEW, D], mybir.dt.float32)
            post = pool.tile([rpc, n_chunks], mybir.dt.int32)
            bast = pool.tile([rpc, n_chunks], mybir.dt.int32)
            idxt = pool.tile([rpc, n_chunks], mybir.dt.int32)

            # Stage the 32 new rows and the positions in SBUF.
            nc.gpsimd.dma_start(out=newt[:], in_=newkv[:])
            nc.gpsimd.dma_start(out=post[:], in_=pos[:])

            # idx[q, c] = (c*rpc + q)*S + pos[q, c]: the flat [ROWS, D] row
            # index of scattered row p = c*rpc + q.
            nc.gpsimd.iota(
                bast[:], pattern=[[rpc * S, n_chunks]], base=0, channel_multiplier=S
            )
            nc.vector.tensor_tensor(
                out=idxt[:], in0=bast[:], in1=post[:], op=mybir.AluOpType.add
            )

            prev = None
            for _ in range(chain_k):
                cur = []
                for c in range(n_chunks):
                    sl = slice(c * rows_pc, (c + 1) * rows_pc)
                    eng = nc.sync if c % 2 == 0 else nc.scalar
                    b = eng.dma_start(out=out[sl, :], in_=kv[sl, :])
                    if prev is not None:
                        add_dep_helper(b.ins, prev[c].ins, reason="chain iterations")
                    # Chunk c covers flat rows [c*rows_pc, (c+1)*rows_pc) =
                    # exactly the targets of rows p in [c*rpc, (c+1)*rpc), so
                    # this scatter waits only on its own chunk and overlaps
                    # the remaining copy.
                    # out AP must start at offset 0 (indirect-DMA constraint)
                    # but is narrowed to the prefix ending at this chunk: it
                    # still contains every target row of this scatter, and
                    # Tile's WAW tracking then sees no conflict with LATER
                    # chunk copies — otherwise each copy c+2 is auto-gated on
                    # scatter c's completion (~4-5us cross-engine hop each).
                    sc = nc.gpsimd.indirect_dma_start(
                        out=out[: (c + 1) * rows_pc, :],
                        out_offset=bass.IndirectOffsetOnAxis(
                            ap=idxt[:, c : c + 1], axis=0
                        ),
                        in_=newt[c * rpc : (c + 1) * rpc, :],
                        in_offset=None,
                    )
                    add_dep_helper(
                        sc.ins, b.ins, reason="scatter lands after its copy chunk"
                    )
                    cur.append(sc)
                prev = cur

    nc.compile()
    _NC_CACHE[key] = nc
    return nc


def kernel(kv_cache, new_kv, position_ids):
    kv_cache = np.asarray(kv_cache)
    new_kv = np.asarray(new_kv)
    position_ids = np.asarray(position_ids)

    nc = _build()

    # positions replicated for all 4 (layer', ) groups (row p holds pos[p % 8]),
    # laid out [rpc, n_chunks] column-major-by-chunk: element (q, c) is the
    # position of scattered row p = c*rpc + q (see _build).
    n_chunks = 8
    rpc = NEW // n_chunks
    pos32 = np.tile(position_ids[:, 0].astype(np.int32), LP)
    pos32 = np.ascontiguousarray(pos32.reshape(n_chunks, rpc).T)

    # one strided pass each; per-core shards are then contiguous views
    kv_sh = np.ascontiguousarray(kv_cache.transpose(2, 0, 1, 3, 4))
    new_sh = np.ascontiguousarray(new_kv[:, :, :, :, 0, :].transpose(3, 0, 1, 2, 4))
    in_maps = [
        {
            "kv": kv_sh[h].reshape(ROWS, D),
            "newkv": new_sh[h].reshape(NEW, D),
            "pos": pos32,
        }
        for h in range(NCORES)
    ]

    res = run_bass_kernel_spmd(nc, in_maps, core_ids=list(range(NCORES)))
    outs = [r["out"].reshape(LP, B, S, D) for r in res.results]
    return np.stack(outs, axis=2)



# revision 2
# speedup vs baseline: 74.6527x; 74.6527x over previous
"""KV-cache scatter update kernel for Trainium2 (8 NeuronCores), in-place.

Problem: kv_cache (2L=4, B=8, H=8, S=4096, D=128) f32, new_kv (L=2, 2, B=8,
H=8, 1, D=128) f32, position_ids (B=8, 1) int. Output = kv_cache with
new_kv[l, kv, b, h, 0, :] written at [2l+kv, b, h, pos[b], :].

The output differs from the input kv_cache in only 256 rows (128 KB of
512 MB), so the kernel updates the cache *in place* instead of copying it:
the kv shard is passed to the NEFF as the donated buffer backing the "out"
tensor (exactly where run_bass_via_pjrt passes a donated zero buffer — same
custom-call + donation mechanism, same NEFF execution path). XLA aliases the
donated input to the NEFF output, so the device program only has to scatter
its 32 new rows; every unwritten row keeps its original kv content.

Device program per core (3 instructions):
  - DMA newkv (32 x 128 f32, 16 KB) DRAM->SBUF on the sync engine's queue
  - DMA idx   (32 x 1 int32)        DRAM->SBUF on the scalar engine's queue
  - one indirect DMA (qPoolDynamic) scattering the 32 rows into the donated
    cache buffer at host-precomputed flat row offsets

Sharding: axis 0 of kv_cache viewed as (2L*B, H, S, D) = (32, H, S, D),
4 consecutive (layer', batch) groups per core. Unlike sharding on H, this
makes every host-side shard/unshard step a zero-copy view: the global
device array is kv_cache.reshape(2L*B*H*S, D) itself, and the output is
reshaped back without any transpose.
"""

import numpy as np

import concourse.bacc as bacc
import concourse.bass as bass
import concourse.mybir as mybir
import concourse.tile as tile
from concourse.tile import add_dep_helper

L = 2
B = 8
H = 8
S = 4096
D = 128
NCORES = 8
LP = 2 * L                 # 4 "layers" in the output (k/v interleaved)
GB = LP * B                # 32 global (layer', batch) groups
PCG = GB // NCORES         # 4 groups per core
NEW = PCG * H              # 32 scattered rows per core
ROWS = PCG * H * S         # 131072 flat cache rows of D floats per core

_NC_CACHE = {}
_RUNNER_CACHE = {}


def _build(chain_k: int = 1):
    """Build the Bass module (one NEFF, same program on all 8 cores).

    chain_k > 1 repeats the body (2 loads + scatter) dependency-chained K
    times — used only by the timing harness to measure steady-state
    per-iteration HW time via the slope method (dispatch overhead cancels).
    """
    key = chain_k
    if key in _NC_CACHE:
        return _NC_CACHE[key]

    nc = bacc.Bacc(
        "TRN2", target_bir_lowering=False, debug=False, num_devices=NCORES
    )
    newkv = nc.dram_tensor("newkv", [NEW, D], mybir.dt.float32, kind="ExternalInput")
    idx = nc.dram_tensor("idx", [NEW, 1], mybir.dt.int32, kind="ExternalInput")
    # "out" is backed by the donated kv shard at run time: only the 32
    # scattered rows are written; the rest keeps the original cache data.
    out = nc.dram_tensor("out", [ROWS, D], mybir.dt.float32, kind="ExternalOutput")

    with tile.TileContext(nc) as tc:
        with tc.tile_pool(name="sb", bufs=1) as pool:
            newt = pool.tile([NEW, D], mybir.dt.float32)
            idxt = pool.tile([NEW, 1], mybir.dt.int32)
            prev = None
            for _ in range(chain_k):
                ld_new = nc.sync.dma_start(out=newt[:], in_=newkv[:])
                ld_idx = nc.scalar.dma_start(out=idxt[:], in_=idx[:])
                if prev is not None:
                    add_dep_helper(ld_new.ins, prev.ins, reason="chain iterations")
                    add_dep_helper(ld_idx.ins, prev.ins, reason="chain iterations")
                sc = nc.gpsimd.indirect_dma_start(
                    out=out[:, :],
                    out_offset=bass.IndirectOffsetOnAxis(ap=idxt[:, 0:1], axis=0),
                    in_=newt[:],
                    in_offset=None,
                )
                prev = sc
    nc.compile()
    _NC_CACHE[key] = nc
    return nc


def _make_runner(chain_k: int = 1):
    """Jitted 8-core shard_map runner for the chain_k NEFF.

    Mirrors concourse.bass2jax.run_bass_via_pjrt's multi-core path exactly,
    except the donated buffer backing "out" is caller-supplied (the kv
    shard) instead of zeros.
    """
    if chain_k in _RUNNER_CACHE:
        return _RUNNER_CACHE[chain_k]

    import jax
    import concourse.bass2jax as b2j
    from jax.experimental.shard_map import shard_map
    from jax.sharding import Mesh, NamedSharding, PartitionSpec

    nc = _build(chain_k)
    b2j.install_neuronx_cc_hook()

    partition_name = nc.partition_id_tensor.name if nc.partition_id_tensor else None
    in_names, out_names, out_avals = [], [], []
    for alloc in nc.m.functions[0].allocations:
        if not isinstance(alloc, mybir.MemoryLocationSet):
            continue
        name = alloc.memorylocations[0].name
        if alloc.kind == "ExternalInput":
            if name != partition_name:
                in_names.append(name)
        elif alloc.kind == "ExternalOutput":
            out_names.append(name)
            out_avals.append(
                jax.core.ShapedArray(tuple(alloc.tensor_shape), mybir.dt.np(alloc.dtype))
            )
    n_params = len(in_names)
    all_in_names = list(in_names) + list(out_names)
    if partition_name is not None:
        all_in_names.append(partition_name)

    def _body(*args):
        operands = list(args)
        if partition_name is not None:
            operands.append(b2j.partition_id_tensor())
        outs = b2j._bass_exec_p.bind(
            *operands,
            out_avals=tuple(out_avals),
            in_names=tuple(all_in_names),
            out_names=tuple(out_names),
            lowering_input_output_aliases=(),
            sim_require_finite=True,
            sim_require_nnan=True,
            nc=nc,
        )
        return tuple(outs)

    devices = jax.devices()[:NCORES]
    mesh = Mesh(np.asarray(devices), ("core",))
    sh = NamedSharding(mesh, PartitionSpec("core"))
    n_outs = len(out_names)
    donate = tuple(range(n_params, n_params + n_outs))
    sharded = jax.jit(
        shard_map(
            _body,
            mesh=mesh,
            in_specs=(PartitionSpec("core"),) * (n_params + n_outs),
            out_specs=(PartitionSpec("core"),) * n_outs,
            check_rep=False,
        ),
        donate_argnums=donate,
        keep_unused=True,
    )
    _RUNNER_CACHE[chain_k] = (sharded, in_names, sh)
    return _RUNNER_CACHE[chain_k]


def _host_prep(kv_cache, new_kv, position_ids):
    """Global (concat-across-cores) arrays; kv is a zero-copy view."""
    kv_g = kv_cache.reshape(GB * H * S, D)
    new_g = np.ascontiguousarray(new_kv[:, :, :, :, 0, :]).reshape(GB * H, D)
    pos = position_ids[:, 0].astype(np.int32)  # (B,)
    # per-core row (r, h) -> flat shard row r*H*S + h*S + pos[b], b = (c*PCG+r) % B
    r = np.arange(NCORES * PCG)                         # global group id = c*PCG + r
    base = (r[:, None] % PCG) * (H * S) + np.arange(H)[None, :] * S  # (GB, H)
    idx = (base + pos[r % B][:, None]).astype(np.int32)  # (GB, H)
    return kv_g, new_g, idx.reshape(NCORES * NEW, 1)


def kernel(kv_cache, new_kv, position_ids):
    import jax

    kv_cache = np.asarray(kv_cache)
    new_kv = np.asarray(new_kv)
    position_ids = np.asarray(position_ids)

    sharded, in_names, sh = _make_runner(chain_k=1)
    kv_g, new_g, idx_g = _host_prep(kv_cache, new_kv, position_ids)

    kv_dev = jax.device_put(kv_g, sh)
    by_name = {
        "newkv": jax.device_put(new_g, sh),
        "idx": jax.device_put(idx_g, sh),
    }
    args = [by_name[n] for n in in_names] + [kv_dev]
    (out,) = sharded(*args)
    return np.asarray(out).reshape(LP, B, H, S, D)


# revision 3
# speedup vs baseline: 80.4171x; 1.0772x over previous
"""KV-cache scatter update kernel for Trainium2 (8 NeuronCores), in-place.

Problem: kv_cache (2L=4, B=8, H=8, S=4096, D=128) f32, new_kv (L=2, 2, B=8,
H=8, 1, D=128) f32, position_ids (B=8, 1) int. Output = kv_cache with
new_kv[l, kv, b, h, 0, :] written at [2l+kv, b, h, pos[b], :].

The output differs from the input kv_cache in only 256 rows (128 KB of
512 MB), so the kernel updates the cache *in place* instead of copying it:
the kv shard is passed to the NEFF as the donated buffer backing the "out"
tensor (exactly where run_bass_via_pjrt passes a donated zero buffer — the
same custom-call + donation mechanism and NEFF execution path). XLA aliases
the donated input to the NEFF output, so the device program only has to
scatter its 32 new rows; every unwritten row keeps its original kv content.
This removes the 64 MB/core read + 64 MB/core write of the full-copy
approach (~400 us at the per-core HBM share) entirely.

Device program per core (2 instructions, ~5 us dominated by fixed DMA
latencies: HWDGE gen ~625 ns + DGE-DMA delay 650 ns + completion-semaphore
propagation 900 ns per serial DMA stage):
  - one DMA loads a packed [32, 129] int32 tensor DRAM->SBUF on the sync
    (SP) engine's queue: cols 0..127 are the f32 bits of the 32 new rows,
    col 128 is the flat destination row index (host-precomputed)
  - one indirect DMA (qPoolDynamic) scatters the 32 rows (512 B each) into
    the donated cache buffer at those offsets

All tensors are int32 bit-views of the f32 data (DMAs move bytes; the
result is bit-identical), which lets the index column ride in the same
load as the payload.

Sharding: axis 0 of kv_cache viewed as (2L*B, H, S, D) = (32, H, S, D),
4 consecutive (layer', batch) groups per core. Unlike sharding on H, this
makes every host-side shard/unshard step a zero-copy view: the global
device array is kv_cache.reshape(2L*B*H*S, D) itself, and the output is
reshaped back without any transpose or copy.
"""

import numpy as np

import concourse.bacc as bacc
import concourse.bass as bass
import concourse.mybir as mybir
import concourse.tile as tile
from concourse.tile import add_dep_helper

L = 2
B = 8
H = 8
S = 4096
D = 128
NCORES = 8
LP = 2 * L                 # 4 "layers" in the output (k/v interleaved)
GB = LP * B                # 32 global (layer', batch) groups
PCG = GB // NCORES         # 4 groups per core
NEW = PCG * H              # 32 scattered rows per core
ROWS = PCG * H * S         # 131072 flat cache rows of D floats per core

_NC_CACHE = {}
_RUNNER_CACHE = {}


def _build(chain_k: int = 1):
    """Build the Bass module (one NEFF, same program on all 8 cores).

    chain_k > 1 repeats the body (packed load + indirect scatter)
    dependency-chained K times — used only by the timing harness to measure
    steady-state per-iteration HW time via the slope method (dispatch
    overhead cancels). The chain is fully serial (each load waits on the
    previous scatter), so the slope upper-bounds single-shot body latency.
    """
    if chain_k in _NC_CACHE:
        return _NC_CACHE[chain_k]

    nc = bacc.Bacc(
        "TRN2", target_bir_lowering=False, debug=False, num_devices=NCORES
    )
    packed = nc.dram_tensor(
        "packed", [NEW, D + 1], mybir.dt.int32, kind="ExternalInput"
    )
    # "out" is backed by the donated kv shard at run time: only the 32
    # scattered rows are written; the rest keeps the original cache data.
    out = nc.dram_tensor("out", [ROWS, D], mybir.dt.int32, kind="ExternalOutput")

    with tile.TileContext(nc) as tc:
        with tc.tile_pool(name="sb", bufs=1) as pool:
            t = pool.tile([NEW, D + 1], mybir.dt.int32)
            prev = None
            for _ in range(chain_k):
                ld = nc.sync.dma_start(out=t[:], in_=packed[:])
                if prev is not None:
                    add_dep_helper(ld.ins, prev.ins, reason="chain iterations")
                sc = nc.gpsimd.indirect_dma_start(
                    out=out[:, :],
                    out_offset=bass.IndirectOffsetOnAxis(ap=t[:, D : D + 1], axis=0),
                    in_=t[:, 0:D],
                    in_offset=None,
                )
                prev = sc
    nc.compile()
    _NC_CACHE[chain_k] = nc
    return nc


def _make_runner(chain_k: int = 1):
    """Jitted 8-core shard_map runner for the chain_k NEFF.

    Mirrors concourse.bass2jax.run_bass_via_pjrt's multi-core path exactly,
    except the donated buffer backing "out" is caller-supplied (the kv
    shard) instead of zeros. run_bass_kernel_spmd cannot express this (its
    axon path hardcodes donated zero buffers and ignores aliases=), hence
    the inlined runner; the executed custom call / NEFF is identical.
    """
    if chain_k in _RUNNER_CACHE:
        return _RUNNER_CACHE[chain_k]

    import jax
    import concourse.bass2jax as b2j
    from jax.experimental.shard_map import shard_map
    from jax.sharding import Mesh, NamedSharding, PartitionSpec

    nc = _build(chain_k)
    b2j.install_neuronx_cc_hook()

    partition_name = nc.partition_id_tensor.name if nc.partition_id_tensor else None
    in_names, out_names, out_avals = [], [], []
    for alloc in nc.m.functions[0].allocations:
        if not isinstance(alloc, mybir.MemoryLocationSet):
            continue
        name = alloc.memorylocations[0].name
        if alloc.kind == "ExternalInput":
            if name != partition_name:
                in_names.append(name)
        elif alloc.kind == "ExternalOutput":
            out_names.append(name)
            out_avals.append(
                jax.core.ShapedArray(tuple(alloc.tensor_shape), mybir.dt.np(alloc.dtype))
            )
    n_params = len(in_names)
    all_in_names = list(in_names) + list(out_names)
    if partition_name is not None:
        all_in_names.append(partition_name)

    def _body(*args):
        operands = list(args)
        if partition_name is not None:
            operands.append(b2j.partition_id_tensor())
        outs = b2j._bass_exec_p.bind(
            *operands,
            out_avals=tuple(out_avals),
            in_names=tuple(all_in_names),
            out_names=tuple(out_names),
            lowering_input_output_aliases=(),
            sim_require_finite=True,
            sim_require_nnan=True,
            nc=nc,
        )
        return tuple(outs)

    devices = jax.devices()[:NCORES]
    mesh = Mesh(np.asarray(devices), ("core",))
    sh = NamedSharding(mesh, PartitionSpec("core"))
    n_outs = len(out_names)
    donate = tuple(range(n_params, n_params + n_outs))
    sharded = jax.jit(
        shard_map(
            _body,
            mesh=mesh,
            in_specs=(PartitionSpec("core"),) * (n_params + n_outs),
            out_specs=(PartitionSpec("core"),) * n_outs,
            check_rep=False,
        ),
        donate_argnums=donate,
        keep_unused=True,
    )
    _RUNNER_CACHE[chain_k] = (sharded, in_names, sh)
    return _RUNNER_CACHE[chain_k]


def _host_prep(kv_cache, new_kv, position_ids):
    """Global (concat-across-cores) arrays; kv is a zero-copy bit-view.

    packed[g*H + h] = [bits of new row (g, h) | flat dest row index], where
    g = c*PCG + r is the global (layer', batch) group and the flat index in
    core c's shard is r*H*S + h*S + pos[b], b = g % B.
    """
    kv_g = kv_cache.reshape(GB * H * S, D).view(np.int32)
    new_g = (
        np.ascontiguousarray(new_kv[:, :, :, :, 0, :])
        .reshape(GB * H, D)
        .view(np.int32)
    )
    pos = position_ids[:, 0].astype(np.int32)               # (B,)
    g = np.arange(GB)
    base = (g[:, None] % PCG) * (H * S) + np.arange(H)[None, :] * S  # (GB, H)
    idx = (base + pos[g % B][:, None]).astype(np.int32).reshape(GB * H, 1)
    packed = np.concatenate([new_g, idx], axis=1)           # (GB*H, D+1) int32
    return kv_g, packed


def kernel(kv_cache, new_kv, position_ids):
    import jax

    kv_cache = np.asarray(kv_cache)
    new_kv = np.asarray(new_kv)
    position_ids = np.asarray(position_ids)

    sharded, in_names, sh = _make_runner(chain_k=1)
    assert in_names == ["packed"], in_names
    kv_g, packed = _host_prep(kv_cache, new_kv, position_ids)

    kv_dev = jax.device_put(kv_g, sh)
    pk_dev = jax.device_put(packed, sh)
    (out,) = sharded(pk_dev, kv_dev)
    return np.asarray(out).view(np.float32).reshape(LP, B, H, S, D)


# revision 5
# speedup vs baseline: 80.4815x; 1.0008x over previous
"""KV-cache scatter update kernel for Trainium2 (8 NeuronCores), in-place.

Problem: kv_cache (2L=4, B=8, H=8, S=4096, D=128) f32, new_kv (L=2, 2, B=8,
H=8, 1, D=128) f32, position_ids (B=8, 1) int. Output = kv_cache with
new_kv[l, kv, b, h, 0, :] written at [2l+kv, b, h, pos[b], :].

The output differs from the input kv_cache in only 256 rows (128 KB of
512 MB), so the kernel updates the cache *in place* instead of copying it:
the kv shard is passed to the NEFF as the donated buffer backing the "out"
tensor (exactly where run_bass_via_pjrt passes a donated zero buffer — the
same custom-call + donation mechanism and NEFF execution path). XLA aliases
the donated input to the NEFF output, so the device program only has to
scatter its 32 new rows; every unwritten row keeps its original kv content.
This removes the 64 MB/core read + 64 MB/core write of the full-copy
approach (~400 us at the per-core HBM share) entirely.

Device program per core (2 instructions, ~4.8 us dominated by fixed DMA
latencies: DGE-DMA delay 650 ns + completion-semaphore propagation 900 ns
per serial DMA stage; both stages issue from the Pool engine, whose
sequencer dispatch is ~500 ns cheaper than SP's HWDGE generation — A/B
measured 4.8 us vs 5.4 us):
  - one DMA loads a packed [32, 129] int32 tensor DRAM->SBUF on the Pool
    (gpsimd) engine's queue: cols 0..127 are the f32 bits of the 32 new
    rows, col 128 is the flat destination row index (host-precomputed)
  - one indirect DMA (qPoolDynamic) scatters the 32 rows (512 B each) into
    the donated cache buffer at those offsets

All tensors are int32 bit-views of the f32 data (DMAs move bytes; the
result is bit-identical), which lets the index column ride in the same
load as the payload.

Sharding: axis 0 of kv_cache viewed as (2L*B, H, S, D) = (32, H, S, D),
4 consecutive (layer', batch) groups per core. Unlike sharding on H, this
makes every host-side shard/unshard step a zero-copy view: the global
device array is kv_cache.reshape(2L*B*H*S, D) itself, and the output is
reshaped back without any transpose or copy.
"""

import numpy as np

import concourse.bacc as bacc
import concourse.bass as bass
import concourse.mybir as mybir
import concourse.tile as tile
from concourse.tile import add_dep_helper

L = 2
B = 8
H = 8
S = 4096
D = 128
NCORES = 8
LP = 2 * L                 # 4 "layers" in the output (k/v interleaved)
GB = LP * B                # 32 global (layer', batch) groups
PCG = GB // NCORES         # 4 groups per core
NEW = PCG * H              # 32 scattered rows per core
ROWS = PCG * H * S         # 131072 flat cache rows of D floats per core

_NC_CACHE = {}
_RUNNER_CACHE = {}


def _build(chain_k: int = 1):
    """Build the Bass module (one NEFF, same program on all 8 cores).

    chain_k > 1 repeats the body (packed load + indirect scatter)
    dependency-chained K times — used only by the timing harness to measure
    steady-state per-iteration HW time via the slope method (dispatch
    overhead cancels). The chain is fully serial (each load waits on the
    previous scatter), so the slope upper-bounds single-shot body latency.
    """
    if chain_k in _NC_CACHE:
        return _NC_CACHE[chain_k]

    nc = bacc.Bacc(
        "TRN2", target_bir_lowering=False, debug=False, num_devices=NCORES
    )
    packed = nc.dram_tensor(
        "packed", [NEW, D + 1], mybir.dt.int32, kind="ExternalInput"
    )
    # "out" is backed by the donated kv shard at run time: only the 32
    # scattered rows are written; the rest keeps the original cache data.
    out = nc.dram_tensor("out", [ROWS, D], mybir.dt.int32, kind="ExternalOutput")

    with tile.TileContext(nc) as tc:
        with tc.tile_pool(name="sb", bufs=1) as pool:
            t = pool.tile([NEW, D + 1], mybir.dt.int32)
            prev = None
            for _ in range(chain_k):
                ld = nc.gpsimd.dma_start(out=t[:], in_=packed[:])
                if prev is not None:
                    add_dep_helper(ld.ins, prev.ins, reason="chain iterations")
                sc = nc.gpsimd.indirect_dma_start(
                    out=out[:, :],
                    out_offset=bass.IndirectOffsetOnAxis(ap=t[:, D : D + 1], axis=0),
                    in_=t[:, 0:D],
                    in_offset=None,
                )
                prev = sc
    nc.compile()
    _NC_CACHE[chain_k] = nc
    return nc


def _make_runner(chain_k: int = 1):
    """Jitted 8-core shard_map runner for the chain_k NEFF.

    Mirrors concourse.bass2jax.run_bass_via_pjrt's multi-core path exactly,
    except the donated buffer backing "out" is caller-supplied (the kv
    shard) instead of zeros. run_bass_kernel_spmd cannot express this (its
    axon path hardcodes donated zero buffers and ignores aliases=), hence
    the inlined runner; the executed custom call / NEFF is identical.
    """
    if chain_k in _RUNNER_CACHE:
        return _RUNNER_CACHE[chain_k]

    import jax
    import concourse.bass2jax as b2j
    from jax.experimental.shard_map import shard_map
    from jax.sharding import Mesh, NamedSharding, PartitionSpec

    nc = _build(chain_k)
    b2j.install_neuronx_cc_hook()

    partition_name = nc.partition_id_tensor.name if nc.partition_id_tensor else None
    in_names, out_names, out_avals = [], [], []
    for alloc in nc.m.functions[0].allocations:
        if not isinstance(alloc, mybir.MemoryLocationSet):
            continue
        name = alloc.memorylocations[0].name
        if alloc.kind == "ExternalInput":
            if name != partition_name:
                in_names.append(name)
        elif alloc.kind == "ExternalOutput":
            out_names.append(name)
            out_avals.append(
                jax.core.ShapedArray(tuple(alloc.tensor_shape), mybir.dt.np(alloc.dtype))
            )
    n_params = len(in_names)
    all_in_names = list(in_names) + list(out_names)
    if partition_name is not None:
        all_in_names.append(partition_name)

    def _body(*args):
        operands = list(args)
        if partition_name is not None:
            operands.append(b2j.partition_id_tensor())
        outs = b2j._bass_exec_p.bind(
            *operands,
            out_avals=tuple(out_avals),
            in_names=tuple(all_in_names),
            out_names=tuple(out_names),
            lowering_input_output_aliases=(),
            sim_require_finite=True,
            sim_require_nnan=True,
            nc=nc,
        )
        return tuple(outs)

    devices = jax.devices()[:NCORES]
    mesh = Mesh(np.asarray(devices), ("core",))
    sh = NamedSharding(mesh, PartitionSpec("core"))
    n_outs = len(out_names)
    donate = tuple(range(n_params, n_params + n_outs))
    sharded = jax.jit(
        shard_map(
            _body,
            mesh=mesh,
            in_specs=(PartitionSpec("core"),) * (n_params + n_outs),
            out_specs=(PartitionSpec("core"),) * n_outs,
            check_rep=False,
        ),
        donate_argnums=donate,
        keep_unused=True,
    )
    _RUNNER_CACHE[chain_k] = (sharded, in_names, sh)
    return _RUNNER_CACHE[chain_k]


def _host_prep(kv_cache, new_kv, position_ids):
    """Global (concat-across-cores) arrays; kv is a zero-copy bit-view.

    packed[g*H + h] = [bits of new row (g, h) | flat dest row index], where
    g = c*PCG + r is the global (layer', batch) group and the flat index in
    core c's shard is r*H*S + h*S + pos[b], b = g % B.
    """
    kv_g = kv_cache.reshape(GB * H * S, D).view(np.int32)
    new_g = (
        np.ascontiguousarray(new_kv[:, :, :, :, 0, :])
        .reshape(GB * H, D)
        .view(np.int32)
    )
    pos = position_ids[:, 0].astype(np.int32)               # (B,)
    g = np.arange(GB)
    base = (g[:, None] % PCG) * (H * S) + np.arange(H)[None, :] * S  # (GB, H)
    idx = (base + pos[g % B][:, None]).astype(np.int32).reshape(GB * H, 1)
    packed = np.concatenate([new_g, idx], axis=1)           # (GB*H, D+1) int32
    return kv_g, packed


def kernel(kv_cache, new_kv, position_ids):
    import jax

    kv_cache = np.asarray(kv_cache)
    new_kv = np.asarray(new_kv)
    position_ids = np.asarray(position_ids)

    sharded, in_names, sh = _make_runner(chain_k=1)
    assert in_names == ["packed"], in_names
    kv_g, packed = _host_prep(kv_cache, new_kv, position_ids)

    kv_dev = jax.device_put(kv_g, sh)
    pk_dev = jax.device_put(packed, sh)
    (out,) = sharded(pk_dev, kv_dev)
    return np.asarray(out).view(np.float32).reshape(LP, B, H, S, D)
